# revision 7
# baseline (speedup 1.0000x reference)
"""Trainium2 Bass kernel for CellSizePredictor (v5: batched DMA + quad reduce).

reference:
    average = x[:, :n]; numbers = x[:, n:]
    o = numbers * average**alpha
    out = o @ A + einsum('bi,ij,bj->b', o, B, o) + C

Design (data-parallel over 8 cores, batch shard 8192 rows each):
  * Host pre-transposes each x shard to xT [2048, 8192] fp16; device
    streams feature-major tiles with contiguous DMAs.
  * Host folds the quadratic form into U = triu(B+B^T,1)+diag(B) so
    quad_b = sum_j o_bj (o@U)_bj; PE runs 36 of 64 [128x128x512] fp16
    matmul tiles per batch sub-chunk.
  * Epilogue on idle engines (v4): ACT does z16_j = Identity(p_z+A_j)
    from PSUM with per-partition bias; DVE does only fp16 SBUF 2x ops
    (o-mul, psT=z16*oT, reduction adds); ACT adds C at the end.
  * v5 vs v4 (trace: first real MM at 14us behind serialized 661ns
    DMA issues; HAM cold until 25us; 4 ones-matmuls/sub bunched at
    sup end):
      - x loads batched: one DMA covers two 128-row chunks via a 3D
        access pattern ([128, 2, SUP]) -> 8 instead of 16 issues/sup.
      - U loads in 2 DMAs (chunks 0-1 first so j<=1 matmuls unlock
        early, then chunks 2-7).
      - o-mul on [128, 2*SUP] double tiles (4 DVE ops/sup).
      - psT pair/quad tree on DVE; only 2 ones-matmuls per sub, the
        quad0 group emitted mid-sup (interleaved accumulation groups
        on separate PSUM banks) so reduction MMs spread out.
      - 40 warm-up matmuls bridge the HAM busy-window into the real
        MM stream.
"""
import sys

for _p in ("/opt/trn_rl_repo",):
    if _p not in sys.path:
        sys.path.append(_p)

import numpy as np
from contextlib import ExitStack

import concourse.bass as bass
import concourse.tile as tile
from concourse import bacc, mybir
from concourse.bass_utils import run_bass_kernel_spmd

dt = mybir.dt
F32 = dt.float32
F16 = dt.float16

N_CORES = 8
BATCH = 65536
N = 1024
SHARD = BATCH // N_CORES          # 8192
N_IC = N // 128                   # 8 contraction chunks of 128
SUP = 1024                        # batch rows per load super-chunk
BCH = 512                         # batch rows per compute chunk (matmul N)
N_SUP = SHARD // SUP              # 8
SUB = SUP // BCH                  # 2
N_WARM = 40                       # PE warm-up dummy matmuls


def _build(n_sup: int):
    nc = bacc.Bacc("TRN2", target_bir_lowering=False, debug=False)

    rows = n_sup * SUP
    x_d = nc.dram_tensor("xt", [2 * N, rows], F16, kind="ExternalInput").ap()
    u_d = nc.dram_tensor("u", [N, N], F16, kind="ExternalInput").ap()
    a_d = nc.dram_tensor("a2", [128, N_IC], F32, kind="ExternalInput").ap()
    c_d = nc.dram_tensor("c1", [1, 1], F32, kind="ExternalInput").ap()
    out_d = nc.dram_tensor("out", [rows], F32, kind="ExternalOutput").ap()
    out_2d = out_d.rearrange("(a b) -> a b", a=1)
    # 3D views: chunk-of-128-rows becomes a middle dim (p=partition)
    x_3d = x_d.rearrange("(c p) b -> p c b", p=128)   # [128, 16, rows]
    u_3d = u_d.rearrange("(c p) n -> p c n", p=128)   # [128, 8, N]

    with tile.TileContext(nc) as tc, ExitStack() as ctx:
        consts = ctx.enter_context(tc.tile_pool(name="consts", bufs=1))
        xin = ctx.enter_context(tc.tile_pool(name="xin", bufs=2))
        opool = ctx.enter_context(tc.tile_pool(name="opool", bufs=2))
        zpool = ctx.enter_context(tc.tile_pool(name="zpool", bufs=3))
        ppool = ctx.enter_context(tc.tile_pool(name="ppool", bufs=3))
        qpool = ctx.enter_context(tc.tile_pool(name="qpool", bufs=2))
        ps_z = ctx.enter_context(tc.tile_pool(name="ps_z", bufs=3, space="PSUM"))
        ps_r = ctx.enter_context(tc.tile_pool(name="ps_r", bufs=1, space="PSUM"))

        # ---- PE warm-up: dummy matmuls on a zeroed tile so the HAM
        # busy-window opens the clock gate before real work arrives.
        # Borrows a ps_z buf; recycled by the pool afterwards. ----
        warm16 = consts.tile([128, 64], F16)
        nc.vector.memset(warm16[:], 0.0)
        p_warm = ps_z.tile([128, SUP], F32, tag="pz")
        for _ in range(N_WARM):
            nc.tensor.matmul(p_warm[0:64, 0:64], warm16[:], warm16[:],
                             start=True, stop=True)

        # ---- constants ----
        u_all = consts.tile([128, N_IC * N], F16)
        u_sb = [u_all[:, i * N : (i + 1) * N] for i in range(N_IC)]
        u3 = u_all[:].rearrange("p (c n) -> p c n", c=N_IC)
        a_sb = consts.tile([128, N_IC], F32)
        c_sb = consts.tile([1, 1], F32)
        ones_f = consts.tile([128, 1], F32)
        nc.vector.memset(ones_f[:], 1.0)
        ones_h = consts.tile([128, 1], F16)
        nc.vector.tensor_copy(ones_h[:], ones_f[:])
        out_sb = consts.tile([1, rows], F32)

        for sc in range(n_sup):
            r0 = sc * SUP
            # ---- batched feature-major loads: one DMA per pair of
            # 128-row chunks ([128, 2, SUP] pattern) ----
            avg2, num2 = [], []
            for k in range(N_IC // 2):
                at = xin.tile([128, 2 * SUP], F16, tag=f"avg{k}")
                nc.sync.dma_start(
                    at[:].rearrange("p (c b) -> p c b", c=2),
                    x_3d[:, 2 * k : 2 * k + 2, r0 : r0 + SUP],
                )
                avg2.append(at)
                nt = xin.tile([128, 2 * SUP], F16, tag=f"num{k}")
                nc.sync.dma_start(
                    nt[:].rearrange("p (c b) -> p c b", c=2),
                    x_3d[:, N_IC + 2 * k : N_IC + 2 * k + 2, r0 : r0 + SUP],
                )
                num2.append(nt)
                if sc == 0:
                    if k == 0:
                        # U chunks 0-1 early so j<=1 matmuls unlock
                        nc.sync.dma_start(
                            u_all[:, 0 : 2 * N].rearrange(
                                "p (c n) -> p c n", c=2
                            ),
                            u_3d[:, 0:2, :],
                        )
                    elif k == 1:
                        nc.sync.dma_start(
                            u_all[:, 2 * N :].rearrange(
                                "p (c n) -> p c n", c=N_IC - 2
                            ),
                            u_3d[:, 2:, :],
                        )
                        nc.sync.dma_start(a_sb[:], a_d)
                        nc.sync.dma_start(c_sb[:], c_d)

            # ---- oT = avgT * numT on double tiles (fp16, DVE 2x) ----
            oT = []
            for k in range(N_IC // 2):
                ot = opool.tile([128, 2 * SUP], F16, tag=f"o{k}")
                nc.vector.tensor_mul(ot[:], avg2[k][:], num2[k][:])
                oT.append(ot[:, 0:SUP])
                oT.append(ot[:, SUP : 2 * SUP])

            p_ress = []
            for sub in range(SUB):
                pr = ps_r.tile([1, BCH], F32, tag=f"pres{sub}")
                p_ress.append(pr)
            psTs = []
            pairs = []
            quads = []
            for j in range(N_IC):
                jsl = slice(j * 128, (j + 1) * 128)
                # zT[j] for both sub-chunks: [128, SUP] psum (2 banks),
                # each sub's accumulation group stays within one bank
                p_z = ps_z.tile([128, SUP], F32, tag="pz")
                for sub in range(SUB):
                    zsl = slice(sub * BCH, (sub + 1) * BCH)
                    for i in range(j + 1):
                        nc.tensor.matmul(
                            p_z[:, zsl],
                            u_sb[i][:, jsl],
                            oT[i][:, zsl],
                            start=(i == 0),
                            stop=(i == j),
                        )
                # z16 = (zT + A_j) on the Scalar engine (PSUM -> SBUF fp16,
                # per-partition bias)
                z16 = zpool.tile([128, SUP], F16, tag="z16")
                nc.scalar.activation(
                    z16[:],
                    p_z[:],
                    mybir.ActivationFunctionType.Identity,
                    bias=a_sb[:, j : j + 1],
                )
                # psT = z16 * oT_j (fp16 SBUF, DVE 2x mode)
                psT = ppool.tile([128, SUP], F16, tag="psT")
                nc.vector.tensor_mul(psT[:], z16[:], oT[j][:])
                psTs.append(psT)
                if j % 2 == 1:
                    pp = qpool.tile([128, SUP], F16, tag=f"pair{(j // 2) % 2}")
                    nc.vector.tensor_add(pp[:], psTs[j - 1][:], psTs[j][:])
                    pairs.append(pp)
                if j % 4 == 3:
                    q = qpool.tile([128, SUP], F16, tag=f"quad{j // 4}")
                    nc.vector.tensor_add(q[:], pairs[-2][:], pairs[-1][:])
                    quads.append(q)
                    # emit this quad's reduction matmuls now (own PSUM
                    # bank, interleaves with later j accumulation groups)
                    for sub in range(SUB):
                        zsl = slice(sub * BCH, (sub + 1) * BCH)
                        nc.tensor.matmul(
                            p_ress[sub][:],
                            ones_h[:],
                            q[:, zsl],
                            start=(j // 4 == 0),
                            stop=(j // 4 == 1),
                        )
            for sub in range(SUB):
                b0 = r0 + sub * BCH
                # final +C on the Scalar engine
                nc.scalar.activation(
                    out_sb[0:1, b0 : b0 + BCH],
                    p_ress[sub][:],
                    mybir.ActivationFunctionType.Identity,
                    bias=c_sb[0:1, 0:1],
                )
            nc.sync.dma_start(
                out_2d[0:1, r0 : r0 + SUP], out_sb[0:1, r0 : r0 + SUP]
            )

    nc.compile()
    return nc


_CACHE: dict = {}


def _get_program(n_sup: int):
    if n_sup not in _CACHE:
        _CACHE[n_sup] = _build(n_sup)
    return _CACHE[n_sup]


def kernel(x, A, B, C, alpha, _n_sup=N_SUP, _trace=False):
    x = np.asarray(x, dtype=np.float32)
    A = np.asarray(A, dtype=np.float32)
    B = np.asarray(B, dtype=np.float32)
    C = np.asarray(C, dtype=np.float32).reshape(-1)
    alpha = np.asarray(alpha, dtype=np.float32)
    assert x.shape == (BATCH, 2 * N), x.shape

    if not np.all(alpha == 1.0):
        # Fallback (setup_inputs always produces alpha == 1): numpy eval.
        o = x[:, N:] * np.power(x[:, :N], alpha[None, :])
        return (o @ A + np.einsum("bi,ij,bj->b", o, B, o) + C[0]).astype(
            np.float32
        )

    nc = _get_program(_n_sup)

    U = np.triu(B + B.T, 1) + np.diag(np.diag(B))
    U16 = U.astype(np.float16)
    x16 = x.astype(np.float16)
    A2 = np.empty((128, N_IC), dtype=np.float32)
    for j in range(N_IC):
        A2[:, j] = A[j * 128 : (j + 1) * 128]
    C1 = np.array([[float(C[0])]], dtype=np.float32)

    rows = _n_sup * SUP
    in_maps = []
    for c in range(N_CORES):
        shard_t = np.ascontiguousarray(x16[c * SHARD : c * SHARD + rows].T)
        in_maps.append({"xt": shard_t, "u": U16, "a2": A2, "c1": C1})
    res = run_bass_kernel_spmd(
        nc, in_maps, list(range(N_CORES)), trace=_trace
    )
    if _trace:
        kernel._last_results = res
    out = np.empty(N_CORES * rows, dtype=np.float32)
    for c in range(N_CORES):
        out[c * rows : (c + 1) * rows] = res.results[c]["out"]
    if rows == SHARD:
        return out
    full = np.zeros(BATCH, dtype=np.float32)
    for c in range(N_CORES):
        full[c * SHARD : c * SHARD + rows] = out[c * rows : (c + 1) * rows]
    return full
